# revision 1
# baseline (speedup 1.0000x reference)
"""CharRNNEmbedder (bidirectional LSTM over char embeddings) on 8 TRN2 cores.

Strategy (data-parallel, 32 sequences/core, fw+bw as two independent chains):
  - Host precomputes G[d] = embed_table @ W[d][:E] + b[d] (+1 on forget gate),
    a 256x512 table: the entire embedding lookup + input-side matmul collapses
    into a per-char gather from G, realized on device as one-hot matmuls.
  - Layout: partitions = H (128); state hT/cT are [128, 32] per direction.
  - Per 4-step window, per dir: one-hot(char) matmuls (2 chunks x 4 gates)
    prefill a PSUM bank with gate pre-activations (start=True); per step the
    4 recurrence matmuls (lhsT = Wh gate slice, rhs = hT) accumulate on top.
  - ACT: tanh(j) + sigmoid(i,f,o) from PSUM, tanh(c) from SBUF (same table set).
  - DVE: cell update.  GPSIMD: snapshot h into hout where t == len-1
    (recurrence itself is unmasked; only the snapshot at len-1 matters).
"""

import numpy as np

B, T, NCHARS, E, H = 256, 512, 256, 256, 128
NCORES = 8
BLOC = B // NCORES  # 32 sequences per core
WIN = 4  # steps per PSUM gather window

_cache = {}


def _build(t_steps, dbg=False):
    from contextlib import ExitStack
    import concourse.tile as tile
    from concourse import bacc, mybir

    f32 = mybir.dt.float32
    Alu = mybir.AluOpType
    Act = mybir.ActivationFunctionType

    nc = bacc.Bacc("TRN2", target_bir_lowering=False, debug=False,
                   num_devices=NCORES)
    N = t_steps * BLOC
    chars_f = nc.dram_tensor("chars_f", [2, N], f32, kind="ExternalInput")
    g_tabs = nc.dram_tensor("g_tabs", [2, 2, 4, 128, 128], f32,
                            kind="ExternalInput")
    wh = nc.dram_tensor("wh", [2, 4, 128, 128], f32, kind="ExternalInput")
    misc = nc.dram_tensor("misc", [128, 2 + BLOC], f32, kind="ExternalInput")
    hout_d = nc.dram_tensor("hout", [2, 128, BLOC], f32,
                            kind="ExternalOutput")
    if dbg:
        z0_d = nc.dram_tensor("z0d", [2, 128, WIN, 4, BLOC], f32,
                              kind="ExternalOutput")
        h_d = nc.dram_tensor("hd", [t_steps, 2, 128, BLOC], f32,
                             kind="ExternalOutput")

    nwin = t_steps // WIN
    with tile.TileContext(nc) as tc, ExitStack() as ctx:
        const = ctx.enter_context(tc.tile_pool(name="const", bufs=1))
        state = ctx.enter_context(tc.tile_pool(name="state", bufs=1))
        work = ctx.enter_context(tc.tile_pool(name="work", bufs=3))
        ohp = ctx.enter_context(tc.tile_pool(name="ohp", bufs=3))
        zp = [ctx.enter_context(tc.tile_pool(name=f"z{d}", bufs=2,
                                             space="PSUM")) for d in (0, 1)]

        # --- constants ---
        gt = [[[const.tile([128, 128], f32, tag=f"gt{d}{c}{g}", name=f"gt{d}{c}{g}")
                for g in range(4)] for c in range(2)] for d in range(2)]
        wt = [[const.tile([128, 128], f32, tag=f"wt{d}{g}", name=f"wt{d}{g}")
               for g in range(4)] for d in range(2)]
        for d in range(2):
            for c in range(2):
                for g in range(4):
                    nc.sync.dma_start(gt[d][c][g][:], g_tabs.ap()[d, c, g])
            for g in range(4):
                nc.sync.dma_start(wt[d][g][:], wh.ap()[d, g])
        mt = const.tile([128, 2 + BLOC], f32, tag="misc", name="misc_t")
        nc.sync.dma_start(mt[:], misc.ap())
        iota = [mt[:, c:c + 1] for c in (0, 1)]
        len_rep = mt[:, 2:2 + BLOC]

        # --- state ---
        h = [state.tile([128, BLOC], f32, tag=f"h{d}", name=f"h{d}") for d in range(2)]
        c = [state.tile([128, BLOC], f32, tag=f"c{d}", name=f"c{d}") for d in range(2)]
        ho = [state.tile([128, BLOC], f32, tag=f"ho{d}", name=f"ho{d}") for d in range(2)]
        for d in range(2):
            nc.vector.memset(h[d][:], 0.0)
            nc.vector.memset(c[d][:], 0.0)
            nc.gpsimd.memset(ho[d][:], 0.0)

        # gather window: one-hot MMs prefill psum [128, WIN, 4, 32]
        # (free layout: t-major, then gate, then batch)
        def gather(w):
            ztiles = []
            for d in range(2):
                z = zp[d].tile([128, WIN, 4, BLOC], f32, tag=f"zw{d}", name=f"zw{d}")
                rep = ohp.tile([128, WIN * BLOC], f32, tag=f"rep{d}", name=f"rep{d}")
                src = chars_f.ap()[d:d + 1, w * WIN * BLOC:(w + 1) * WIN * BLOC]
                nc.sync.dma_start(rep[:], src.partition_broadcast(128))
                for ci in range(2):
                    oh = ohp.tile([128, WIN * BLOC], f32, tag=f"oh{d}{ci}", name=f"oh{d}{ci}")
                    nc.vector.tensor_scalar(oh[:], rep[:], iota[ci], None,
                                            Alu.is_equal)
                    for g in range(4):
                        # out columns (t, b) for gate g
                        nc.tensor.matmul(
                            z[:, :, g, :], gt[d][ci][g][:], oh[:],
                            start=(ci == 0 and g == 0), stop=False,
                            skip_group_check=True)
                ztiles.append(z)
            return ztiles

        def step(zt, t, tw):
            for d in range(2):
                z = zt[d]
                # recurrence matmuls accumulate onto gathered pre-activations
                for g in range(4):
                    last = g == 3 and tw == WIN - 1
                    nc.tensor.matmul(z[:, tw, g, :], wt[d][g][:], h[d][:],
                                     start=False, stop=last,
                                     skip_group_check=True)
                tj = work.tile([128, BLOC], f32, tag=f"tj{d}", name=f"tj{d}")
                sif = work.tile([128, 3, BLOC], f32, tag=f"sif{d}", name=f"sif{d}")
                nc.scalar.activation(tj[:], z[:, tw, 0, :], Act.Tanh)
                nc.scalar.activation(sif[:], z[:, tw, 1:4, :], Act.Sigmoid)
                p1 = work.tile([128, BLOC], f32, tag=f"p1{d}", name=f"p1{d}")
                p2 = work.tile([128, BLOC], f32, tag=f"p2{d}", name=f"p2{d}")
                tc_ = work.tile([128, BLOC], f32, tag=f"tc{d}", name=f"tc{d}")
                nc.vector.tensor_mul(p1[:], sif[:, 0, :], tj[:])   # i*jt
                nc.vector.tensor_mul(p2[:], sif[:, 1, :], c[d][:])  # f*c
                nc.vector.tensor_add(c[d][:], p1[:], p2[:])
                nc.scalar.activation(tc_[:], c[d][:], Act.Tanh)
                nc.vector.tensor_mul(h[d][:], tc_[:], sif[:, 2, :])  # o*tanh(c)
                # snapshot h where len == t+1
                dh = work.tile([128, BLOC], f32, tag=f"dh{d}", name=f"dh{d}")
                nc.vector.scalar_tensor_tensor(
                    dh[:], len_rep, float(t + 1), h[d][:],
                    Alu.is_equal, Alu.mult)
                nc.gpsimd.tensor_add(ho[d][:], ho[d][:], dh[:])
                if dbg:
                    nc.sync.dma_start(h_d.ap()[t, d], h[d][:])

        zt = gather(0)
        if dbg:
            for d in range(2):
                zs = work.tile([128, WIN, 4, BLOC], f32, tag=f"zs{d}", name=f"zs{d}")
                nc.vector.tensor_copy(zs[:], zt[d][:])
                nc.sync.dma_start(z0_d.ap()[d], zs[:])
        for w in range(nwin):
            zt_next = gather(w + 1) if w + 1 < nwin else None
            for tw in range(WIN):
                step(zt, w * WIN + tw, tw)
            zt = zt_next

        for d in range(2):
            nc.sync.dma_start(hout_d.ap()[d], ho[d][:])

    nc.compile()
    return nc


def _prep(chars, length, embed_table, Wf, bf, Wb, bb, t_steps):
    """Host-side input prep: weight-derived tables + per-core index shards."""
    perm = np.r_[128:256, 0:128, 256:384, 384:512]  # gate order j,i,f,o
    g_tabs = np.zeros((2, 2, 4, 128, 128), np.float32)
    whx = np.zeros((2, 4, 128, 128), np.float32)
    for d, (W, bias) in enumerate(((Wf, bf), (Wb, bb))):
        G = embed_table.astype(np.float64) @ W[:E].astype(np.float64)
        G = G + bias.astype(np.float64)
        G[:, 256:384] += 1.0  # forget_bias on f gate (TF order cols 256:384)
        G = G[:, perm].astype(np.float32)
        Wh = np.ascontiguousarray(W[E:, perm].astype(np.float32))
        for ci in range(2):
            for g in range(4):
                g_tabs[d, ci, g] = G[ci * 128:(ci + 1) * 128,
                                     g * 128:(g + 1) * 128]
        for g in range(4):
            whx[d, g] = Wh[:, g * 128:(g + 1) * 128]

    tt = np.arange(t_steps)
    rev = np.clip(length[:, None].astype(np.int64) - 1 - tt[None, :], 0,
                  chars.shape[1] - 1)
    chars_bw = np.take_along_axis(np.asarray(chars, np.int64), rev, axis=1)

    ins = []
    for i in range(NCORES):
        sl = slice(i * BLOC, (i + 1) * BLOC)
        cf = np.stack([
            np.asarray(chars[sl, :t_steps], np.float32).T.reshape(-1),
            np.asarray(chars_bw[sl, :t_steps], np.float32).T.reshape(-1),
        ]).astype(np.float32)
        misc = np.zeros((128, 2 + BLOC), np.float32)
        misc[:, 0] = np.arange(128)
        misc[:, 1] = np.arange(128, 256)
        misc[:, 2:] = np.asarray(length[sl], np.float32)[None, :]
        ins.append(dict(chars_f=np.ascontiguousarray(cf),
                        g_tabs=g_tabs, wh=whx,
                        misc=np.ascontiguousarray(misc)))
    return ins


def _run(inputs, t_steps, trace=False):
    from concourse.bass_utils import run_bass_kernel_spmd
    if t_steps not in _cache:
        _cache[t_steps] = _build(t_steps)
    nc = _cache[t_steps]
    ins = _prep(inputs["chars"], inputs["length"], inputs["embed_table"],
                inputs["Wf"], inputs["bf"], inputs["Wb"], inputs["bb"],
                t_steps)
    res = run_bass_kernel_spmd(nc, ins, core_ids=list(range(NCORES)),
                               trace=trace)
    out = np.zeros((B, 2 * H), np.float32)
    for i, r in enumerate(res.results):
        sl = slice(i * BLOC, (i + 1) * BLOC)
        out[sl, :H] = r["hout"][0].T
        out[sl, H:] = r["hout"][1].T
    return out, res


def kernel(chars, length, embed_table, Wf, bf, Wb, bb):
    out, _ = _run(dict(chars=chars, length=length, embed_table=embed_table,
                       Wf=Wf, bf=bf, Wb=Wb, bb=bb), T)
    return out

